# revision 1
# baseline (speedup 1.0000x reference)
"""Multi-head attention (B=2, N=4096, C=768, H=12, RoPE) on 8 trn2 NeuronCores.

Software-pipelined restructure of the phase-sequential baseline. Same math/sharding
(core c owns batch c//4, heads [(c%4)*3, +3)), but the per-head phases are
interleaved so the ScalarE exp stream (the per-core floor: 3*N^2 elems)
runs continuously while TensorE does qkv projections / PV / output
projection for neighboring heads in its slack:

  - attention processes ONE 512-wide q-block per pass (8 passes/head);
    scores pair adjacent k-tiles into PE row-groups (0,0)/(64,0) via the
    duplicated Q/K rows, exp runs in [128,1536] PSUM chunks (2 in flight),
    PV accumulates M=65 (V plus ones column -> softmax denominators in
    PSUM row 64) into a single pvb bank.
  - PSUM budget: 6 banks exp chunks + 1 bank pvb + 1 "work" bank shared
    by qk-projection / V-projection / rb-broadcast / output-projection.
  - background units (next head's qk proj + RoPE, next rep's V proj,
    output projection of finished columns) are emitted paced between
    chunks, so the Tile scheduler overlaps them with the exp stream.
  - QT/KTt and V_all are double-buffered across heads/reps so there are
    no write-after-read false deps between adjacent pipeline stages.

The attention scale 1/sqrt(HD) is folded into the q weights/biases
host-side, so a single fp32 RoPE table serves q and k.

Measured (KREPEAT R=17 delta, median of per-round deltas): ~620 us per
execution (phase-sequential baseline re-measures ~650-670 us under the
same protocol; its original checkpoint number was 891 us); relative
error vs the fp32 jax reference: 7.6e-3 (max-abs / absmax).

HW gotcha found while building this: custom DVE ops (reciprocal_approx_*)
silently return garbage on hardware when an operand's base partition is
nonzero (CoreSim models them fine) — copy the row to partition 0 first.
"""

import os
import sys

sys.path.insert(0, "/opt/trn_rl_repo")

import numpy as np
import ml_dtypes

B, N, C = 2, 4096, 768
H = 12
HD = 64
HH = HD // 2  # 32
THETA = 10000.0
NCORES = 8
HPC = 3  # heads per core
NT = N // 128  # 32 n-tiles
NBLK = N // 512  # 8 q blocks
KT = N // 128  # 32 k-tiles

BF16 = ml_dtypes.bfloat16

_BUILT = {}


def _rope_tables():
    inv = 1.0 / (THETA ** (np.arange(0, HD, 2, dtype=np.float64) / HD))  # [32]
    freqs = np.arange(N, dtype=np.float64)[:, None] * inv[None, :]  # [N, 32]
    cos = np.concatenate([np.cos(freqs), np.cos(freqs)], axis=-1)  # [N, 64]
    sin = np.concatenate([np.sin(freqs), np.sin(freqs)], axis=-1)
    cosT = cos.T.astype(np.float32)  # [64, N]
    sinT = sin.T.astype(np.float32)
    # sinx rows 0:32 = -sin rows 0:32 ; rows 32:64 = +sin rows 32:64
    sinxT = np.concatenate([-sinT[:HH], sinT[HH:]], axis=0)
    return cosT, sinxT


def _host_inputs(x, w_qkv, b_qkv, w_proj, b_proj):
    """Build the per-core input maps (all numpy, fp32/bf16)."""
    x = np.asarray(x, dtype=np.float32)
    w_qkv = np.asarray(w_qkv, dtype=np.float32)
    b_qkv = np.asarray(b_qkv, dtype=np.float32)
    w_proj = np.asarray(w_proj, dtype=np.float32)
    b_proj = np.asarray(b_proj, dtype=np.float32)

    cosT, sinxT = _rope_tables()
    scale = HD ** -0.5
    tab = np.concatenate([cosT, sinxT], axis=0).astype(np.float32)  # [128, N]

    perm = np.concatenate([np.arange(HH, HD), np.arange(0, HH)])  # rotate_half
    wT = w_qkv.T  # [C, 3C]  (c, e)
    wpT = w_proj.T  # [C, C]  (c, dd)

    in_maps = []
    for core in range(NCORES):
        b = core // 4
        h0 = (core % 4) * HPC
        xT = np.ascontiguousarray(x[b].T).astype(BF16)  # [C, N]

        # wqkT: per head two e-tiles of 128: [q(64); qrot(64)], [k(64); krot(64)]
        # the attention scale is folded into the q columns.
        etiles = []
        for h in range(h0, h0 + HPC):
            wq = wT[:, h * HD:(h + 1) * HD] * scale  # [C, 64]
            wk = wT[:, C + h * HD: C + (h + 1) * HD]
            etiles.append(np.concatenate([wq, wq[:, perm]], axis=1))
            etiles.append(np.concatenate([wk, wk[:, perm]], axis=1))
        wqkT = np.ascontiguousarray(np.concatenate(etiles, axis=1)).astype(BF16)

        # v weights, padded to 256 free
        wv = np.concatenate(
            [wT[:, 2 * C + h * HD: 2 * C + (h + 1) * HD] for h in range(h0, h0 + HPC)],
            axis=1,
        )  # [768, 192]
        wvT = np.zeros((C, 256), dtype=BF16)
        wvT[:, :192] = wv.astype(BF16)

        # biases (q biases carry the attention scale)
        bq = np.zeros((128, HPC), dtype=np.float32)
        bk = np.zeros((128, HPC), dtype=np.float32)
        for j, h in enumerate(range(h0, h0 + HPC)):
            bqh = b_qkv[h * HD:(h + 1) * HD] * scale
            bkh = b_qkv[C + h * HD: C + (h + 1) * HD]
            bq[:HD, j] = bqh
            bq[HD:, j] = bqh[perm]
            bk[:HD, j] = bkh
            bk[HD:, j] = bkh[perm]
        bv = np.zeros((128, 256), dtype=np.float32)
        for j, h in enumerate(range(h0, h0 + HPC)):
            bv[:, j * HD:(j + 1) * HD] = b_qkv[2 * C + h * HD: 2 * C + (h + 1) * HD]

        # proj weights: heads A,B stacked; head C + bias ones-row
        hA, hB, hC = h0, h0 + 1, h0 + 2
        wp_ab = np.concatenate(
            [wpT[hA * HD:(hA + 1) * HD], wpT[hB * HD:(hB + 1) * HD]], axis=0
        ).astype(BF16)  # [128, 768]
        wp_c = np.zeros((128, C), dtype=np.float32)
        wp_c[:HD] = wpT[hC * HD:(hC + 1) * HD]
        if core % 4 == 0:
            wp_c[HD] = b_proj  # bias once per batch (summed over 4 cores)
        wp_c = wp_c.astype(BF16)

        in_maps.append(
            {
                "xT": xT,
                "wqkT": wqkT,
                "wvT": wvT,
                "bq": bq,
                "bk": bk,
                "bv": bv,
                "wp_ab": np.ascontiguousarray(wp_ab),
                "wp_c": np.ascontiguousarray(wp_c),
                "tab": np.ascontiguousarray(tab),
                "ones_f": np.ones((1, 64), dtype=np.float32),
            }
        )
    return in_maps


def _build_nc():
    import concourse.bass as bass  # noqa: F401
    import concourse.bacc as bacc
    import concourse.tile as tile
    import concourse.mybir as mybir

    f32 = mybir.dt.float32
    bf16 = mybir.dt.bfloat16

    nc = bacc.Bacc("TRN2", num_devices=NCORES, debug=False)

    xT_d = nc.dram_tensor("xT", [C, N], bf16, kind="ExternalInput").ap()
    wqkT_d = nc.dram_tensor("wqkT", [C, 768], bf16, kind="ExternalInput").ap()
    wvT_d = nc.dram_tensor("wvT", [C, 256], bf16, kind="ExternalInput").ap()
    bq_d = nc.dram_tensor("bq", [128, HPC], f32, kind="ExternalInput").ap()
    bk_d = nc.dram_tensor("bk", [128, HPC], f32, kind="ExternalInput").ap()
    bv_d = nc.dram_tensor("bv", [128, 256], f32, kind="ExternalInput").ap()
    wpab_d = nc.dram_tensor("wp_ab", [128, C], bf16, kind="ExternalInput").ap()
    wpc_d = nc.dram_tensor("wp_c", [128, C], bf16, kind="ExternalInput").ap()
    tab_d = nc.dram_tensor("tab", [128, N], f32, kind="ExternalInput").ap()
    onesf_d = nc.dram_tensor("ones_f", [1, 64], f32, kind="ExternalInput").ap()
    out_d = nc.dram_tensor("out", [N, C], f32, kind="ExternalOutput").ap()
    taps = {}
    if os.environ.get("KTAPS") == "1":
        taps["qt"] = nc.dram_tensor("tap_qt", [128, N], bf16, kind="ExternalOutput").ap()
        taps["kt"] = nc.dram_tensor("tap_kt", [128, N], bf16, kind="ExternalOutput").ap()
        taps["v"] = nc.dram_tensor("tap_v", [128, NT * HPC * (HD + 1)], bf16, kind="ExternalOutput").ap()
        taps["sp"] = nc.dram_tensor("tap_sp", [128, N], bf16, kind="ExternalOutput").ap()
        taps["sc"] = nc.dram_tensor("tap_sc", [128, N], bf16, kind="ExternalOutput").ap()
        taps["e"] = nc.dram_tensor("tap_e", [128, 1536], bf16, kind="ExternalOutput").ap()
        taps["z"] = nc.dram_tensor("tap_z", [128, 512], f32, kind="ExternalOutput").ap()

    with tile.TileContext(nc) as tc:
        _emit(tc, nc, mybir, locals())
    nc.compile()
    return nc


def _emit(tc, nc, mybir, T):
    taps = T["taps"]
    f32 = mybir.dt.float32
    f32r = mybir.dt.float32r
    bf16 = mybir.dt.bfloat16
    ALU = mybir.AluOpType
    EXP = mybir.ActivationFunctionType.Exp

    xT_d = T["xT_d"]; wqkT_d = T["wqkT_d"]; wvT_d = T["wvT_d"]
    bq_d = T["bq_d"]; bk_d = T["bk_d"]; bv_d = T["bv_d"]
    wpab_d = T["wpab_d"]; wpc_d = T["wpc_d"]
    tab_d = T["tab_d"]; onesf_d = T["onesf_d"]; out_d = T["out_d"]

    from contextlib import ExitStack

    ctx = ExitStack()
    with ctx:
        const = ctx.enter_context(tc.tile_pool(name="const", bufs=1))
        ropes = ctx.enter_context(tc.tile_pool(name="ropes", bufs=2))
        norms = ctx.enter_context(tc.tile_pool(name="norms", bufs=2))
        epool = ctx.enter_context(tc.tile_pool(name="epool", bufs=4))
        fout = ctx.enter_context(tc.tile_pool(name="fout", bufs=2))
        hw = ctx.enter_context(tc.tile_pool(name="hw", bufs=1))
        # PSUM: 6 banks chunks + 1 bank pvb + 1 bank work
        scps = ctx.enter_context(tc.tile_pool(name="scps", bufs=2, space="PSUM"))
        pvps = ctx.enter_context(tc.tile_pool(name="pvps", bufs=1, space="PSUM"))
        wkps = ctx.enter_context(tc.tile_pool(name="wkps", bufs=1, space="PSUM"))

        # ---- constants / resident tensors ----
        xT = const.tile([128, 6, N], bf16, tag="xT")
        for ct in range(6):
            nc.sync.dma_start(out=xT[:, ct, :], in_=xT_d[ct * 128:(ct + 1) * 128, :])
        tab = const.tile([128, N], f32, tag="tab")
        nc.sync.dma_start(out=tab, in_=tab_d)
        wvT = const.tile([128, 6, 256], bf16, tag="wvT")
        nc.sync.dma_start(out=wvT, in_=wvT_d.rearrange("(t p) e -> p t e", p=128))
        bv = const.tile([128, 256], f32, tag="bv")
        nc.sync.dma_start(out=bv, in_=bv_d)
        bq = const.tile([128, HPC], f32, tag="bq")
        nc.sync.dma_start(out=bq, in_=bq_d)
        bk = const.tile([128, HPC], f32, tag="bk")
        nc.sync.dma_start(out=bk, in_=bk_d)
        wp_ab = const.tile([128, C], bf16, tag="wp_ab")
        nc.sync.dma_start(out=wp_ab, in_=wpab_d)
        wp_c = const.tile([128, C], bf16, tag="wp_c")
        nc.sync.dma_start(out=wp_c, in_=wpc_d)
        ones_f = const.tile([1, 64], f32, tag="ones_f")
        nc.sync.dma_start(out=ones_f, in_=onesf_d)
        ones_r = const.tile([1, 64], f32r, tag="ones_r")
        nc.vector.tensor_copy(ones_r, ones_f)

        # double-buffered per-head V (ones column at [..., 64] -> softmax
        # denominators free in PSUM row 64 of the PV accumulation)
        V_all = const.tile([128, 2, NT, HPC, HD + 1], bf16, tag="V_all")
        for vb in range(2):
            for j in range(HPC):
                nc.vector.memset(V_all[:, vb, :, j, HD:HD + 1], 1.0)
        stagingP = const.tile([128, N], bf16, tag="stagingP")
        stagingC = const.tile([128, N], bf16, tag="stagingC")
        nc.vector.memset(stagingC[HD:HD + 1, :], 1.0)  # proj-bias ones row

        # double-buffered Q^T/K^T (rows 64:128 duplicate rows 0:64)
        QT = const.tile([128, 2, N], bf16, tag="QT")
        KTt = const.tile([128, 2, N], bf16, tag="KTt")

        wqk_bufs = [
            hw.tile([128, 6, 256], bf16, tag="wqk", name="wqk0"),
            hw.tile([128, 6, 256], bf16, tag="wqk2", name="wqk1"),
        ]

        # ---------------- unit emitters ----------------
        def unit_wqk_load(gh):
            """DMA the qk weights for global-head gh into its buffer."""
            h = gh % HPC
            nc.sync.dma_start(
                out=wqk_bufs[gh % 2],
                in_=wqkT_d.rearrange("(t p) e -> p t e", p=128)[
                    :, :, h * 256:(h + 1) * 256
                ],
            )

        def unit_v(vb, nt):
            """V projection for n-tile nt into V buffer vb."""
            vp = wkps.tile([128, 512], f32, tag="wk", name="vp")
            for ct in range(6):
                nc.tensor.matmul(
                    vp[:, 0:256],
                    lhsT=xT[:, ct, nt * 128:(nt + 1) * 128],
                    rhs=wvT[:, ct, :],
                    start=(ct == 0),
                    stop=(ct == 5),
                )
            nc.vector.scalar_tensor_tensor(
                out=V_all[:, vb, nt, :, 0:HD],
                in0=vp[:, 0:192].rearrange("p (a b) -> p a b", a=HPC),
                scalar=1.0,
                in1=bv[:, 0:192].rearrange("p (a b) -> p a b", a=HPC),
                op0=ALU.mult,
                op1=ALU.add,
            )

        def unit_qk(gh, blk, et):
            """qkv^T projection + RoPE for (head gh, block blk, q/k=et)."""
            h = gh % HPC
            hb = gh % 2
            wqk = wqk_bufs[gh % 2]
            dest = QT[:, hb, :] if et == 0 else KTt[:, hb, :]
            bias = bq if et == 0 else bk
            ns = slice(blk * 512, (blk + 1) * 512)
            qp = wkps.tile([128, 512], f32, tag="wk", name="qp")
            for ct in range(6):
                nc.tensor.matmul(
                    qp,
                    lhsT=wqk[:, ct, et * 128:(et + 1) * 128],
                    rhs=xT[:, ct, ns],
                    start=(ct == 0),
                    stop=(ct == 5),
                )
            t_s = ropes.tile([HD, 512], f32, tag="t_s")
            nc.vector.scalar_tensor_tensor(
                out=t_s,
                in0=qp[HD:128, :],
                scalar=bias[HD:128, h:h + 1],
                in1=tab[HD:128, ns],
                op0=ALU.add,
                op1=ALU.mult,
            )
            u_s = ropes.tile([HD, 512], f32, tag="u_s")
            nc.vector.scalar_tensor_tensor(
                out=u_s,
                in0=qp[0:HD, :],
                scalar=bias[0:HD, h:h + 1],
                in1=tab[0:HD, ns],
                op0=ALU.add,
                op1=ALU.mult,
            )
            nc.vector.tensor_add(dest[0:HD, ns], t_s, u_s)
            nc.vector.tensor_copy(dest[HD:128, ns], dest[0:HD, ns])

        def unit_out(nt):
            """Output projection for n-tile nt (plus DMA)."""
            nsl = slice(nt * 128, (nt + 1) * 128)
            fs = fout.tile([128, C], f32, tag="fs")
            for o, w in ((0, 512), (512, 256)):
                fp = wkps.tile([128, 512], f32, tag="wk", name="fp")
                nc.tensor.matmul(
                    fp[:, 0:w], lhsT=stagingP[:, nsl], rhs=wp_ab[:, o:o + w],
                    start=True, stop=False,
                )
                nc.tensor.matmul(
                    fp[:, 0:w], lhsT=stagingC[0:HD + 1, nsl],
                    rhs=wp_c[0:HD + 1, o:o + w],
                    start=False, stop=True,
                )
                nc.vector.tensor_copy(fs[:, o:o + w], fp[:, 0:w])
            nc.sync.dma_start(out=out_d[nsl, :], in_=fs)

        # paced background-unit emission
        pending = []

        def pump(frac_done):
            """Emit pending units up to frac_done of the current window."""
            target = int(np.ceil(frac_done * pump.total))
            while pump.emitted < target and pending:
                pending.pop(0)()
                pump.emitted += 1

        def set_window():
            pump.total = max(len(pending), 1)
            pump.emitted = 0

        # ---------------- attention pass ----------------
        # The normalization of pass i is decoupled from the PE critical path:
        # at pass end only a DVE copy drains pvb (PSUM->SBUF raw), and the
        # recip/broadcast/multiply chain is emitted early in pass i+1 so the
        # PE never stalls on the DVE at a pass boundary.
        deferred = []

        def flush_deferred():
            while deferred:
                deferred.pop(0)()

        def norm_tail(gh, qb, pvraw, tap_ok):
            h = gh % HPC
            q0 = slice(qb * 512, (qb + 1) * 512)
            # align the denominator row to partition 0 first: custom DVE ops
            # do not handle partition-base-shifted operands on HW
            zc = norms.tile([1, 512], f32, tag="zc")
            nc.vector.tensor_copy(zc, pvraw[HD:HD + 1, :])
            r0 = norms.tile([1, 512], f32, tag="r0")
            nc.vector.reciprocal_approx_fast(out=r0, in_=zc)
            r0r = norms.tile([1, 512], f32r, tag="r0r")
            nc.vector.tensor_copy(r0r, r0)
            rb = wkps.tile([128, 512], f32, tag="wk", name="rb")
            # f32r operands -> full-rate broadcast matmul (512 free >= 256)
            nc.tensor.matmul(
                rb[0:HD, :], lhsT=ones_r, rhs=r0r, start=True, stop=True,
            )
            rbs = norms.tile([HD, 512], f32, tag="rbs")
            nc.vector.tensor_copy(rbs, rb[0:HD, :])
            if tap_ok and qb == 0 and "z" in taps:
                zt = norms.tile([128, 512], f32, tag="zt")
                nc.vector.memset(zt, 0.0)
                nc.vector.tensor_copy(zt[0:1, :], pvraw[HD:HD + 1, :])
                nc.sync.dma_start(out=taps["z"], in_=zt)
            if h == 0:
                d0 = stagingP[0:HD, q0]
            elif h == 1:
                d0 = stagingP[HD:128, q0]
            else:
                d0 = stagingC[0:HD, q0]
            nc.vector.tensor_mul(d0, pvraw[0:HD, :], rbs)
            if h == HPC - 1:
                for nt in range(4 * qb, 4 * qb + 4):
                    unit_out(nt)

        def attn_pass(gh, qb, tap_ok):
            """One 512-wide q-block of softmax attention for head gh."""
            h = gh % HPC
            hb = gh % 2
            vb = (gh // HPC) % 2
            q0 = slice(qb * 512, (qb + 1) * 512)
            pvb = pvps.tile([128, 512], f32, tag="pv", name="pvb")
            nchunks = (KT + 2) // 3
            for ci in range(nchunks):
                kts = list(range(3 * ci, min(3 * ci + 3, KT)))
                width = 512 * len(kts)
                sc = scps.tile([128, 1536], f32, tag="sc", name="sc")
                for j, kt in enumerate(kts):
                    ksl = slice(kt * 128, (kt + 1) * 128)
                    if j % 2 == 0:
                        nc.tensor.matmul(
                            sc[:, j * 512:(j + 1) * 512],
                            lhsT=KTt[0:HD, hb, ksl], rhs=QT[0:HD, hb, q0],
                            start=True, stop=True,
                        )
                    else:
                        nc.tensor.matmul(
                            sc[:, j * 512:(j + 1) * 512],
                            lhsT=KTt[HD:128, hb, ksl], rhs=QT[HD:128, hb, q0],
                            start=True, stop=True, tile_position=(64, 0),
                        )
                ec = epool.tile([128, 1536], bf16, tag="ec")
                nc.scalar.activation(ec[:, 0:width], sc[:, 0:width], EXP)
                if tap_ok and ci == 0 and "e" in taps:
                    nc.sync.dma_start(out=taps["e"], in_=ec)
                for j, kt in enumerate(kts):
                    nc.tensor.matmul(
                        pvb[0:HD + 1, :],
                        lhsT=V_all[:, vb, kt, h, :],
                        rhs=ec[:, j * 512:(j + 1) * 512],
                        start=(kt == 0), stop=(kt == KT - 1),
                    )
                if ci == 2:
                    flush_deferred()
                pump((qb * nchunks + ci + 1) / (NBLK * nchunks))
            # drain pvb with a raw copy; norm chain runs early next pass
            pvraw = norms.tile([HD + 1, 512], f32, tag="pvraw")
            nc.vector.tensor_copy(pvraw, pvb[0:HD + 1, :])
            deferred.append(lambda: norm_tail(gh, qb, pvraw, tap_ok))

        # ---------------- program ----------------
        NREP = int(os.environ.get("KREPEAT", "1"))

        # cold start: weights for head 0, K tiles, V buffer 0, Q block 0
        unit_wqk_load(0)
        for blk in range(NBLK):
            unit_qk(0, blk, 1)  # K tiles
        for nt in range(NT):
            unit_v(0, nt)
        unit_qk(0, 0, 0)  # Q block 0
        unit_wqk_load(1)

        for rep in range(NREP):
            for h in range(HPC):
                gh = rep * HPC + h
                # fill the background queue for this head's window
                pending.clear()
                if rep == 0 and h == 0:
                    # cold start: this head's remaining Q blocks first
                    for blk in range(1, NBLK):
                        pending.append(lambda g=gh, b=blk: unit_qk(g, b, 0))
                if h == 0:
                    # next head's projections (K tiles first, then Q)
                    for blk in range(NBLK):
                        pending.append(lambda g=gh + 1, b=blk: unit_qk(g, b, 1))
                    for blk in range(NBLK):
                        pending.append(lambda g=gh + 1, b=blk: unit_qk(g, b, 0))
                    pending.append(lambda g=gh + 2: unit_wqk_load(g))
                elif h == 1:
                    for blk in range(NBLK):
                        pending.append(lambda g=gh + 1, b=blk: unit_qk(g, b, 1))
                    for blk in range(NBLK):
                        pending.append(lambda g=gh + 1, b=blk: unit_qk(g, b, 0))
                    pending.append(lambda g=gh + 2: unit_wqk_load(g))
                    if rep + 1 < NREP:
                        # next rep's V projection (double-buffered)
                        vb1 = (rep + 1) % 2
                        for nt in range(NT):
                            pending.append(lambda v=vb1, n=nt: unit_v(v, n))
                else:
                    # last head: next rep's head-0 projections; the output
                    # projection interleaves per-q-block below.
                    if rep + 1 < NREP:
                        g1 = (rep + 1) * HPC
                        for blk in range(NBLK):
                            pending.append(lambda g=g1, b=blk: unit_qk(g, b, 1))
                        for blk in range(NBLK):
                            pending.append(lambda g=g1, b=blk: unit_qk(g, b, 0))
                        pending.append(lambda g=g1 + 1: unit_wqk_load(g))
                set_window()

                for qb in range(NBLK):
                    attn_pass(gh, qb, tap_ok=(rep == 0 and h == 0))

            if rep == NREP - 1 and h == HPC - 1:
                flush_deferred()
            if rep == 0 and "qt" in taps:
                nc.sync.dma_start(out=taps["qt"], in_=QT[:, 0, :])
                nc.sync.dma_start(out=taps["kt"], in_=KTt[:, 0, :])
            if rep == 0 and "v" in taps:
                nc.sync.dma_start(
                    out=taps["v"], in_=V_all[:, 0].rearrange("p a b c -> p (a b c)")
                )
            if rep == 0 and "sp" in taps:
                nc.sync.dma_start(out=taps["sp"], in_=stagingP)
                nc.sync.dma_start(out=taps["sc"], in_=stagingC)


def _get_nc():
    if "nc" not in _BUILT:
        _BUILT["nc"] = _build_nc()
    return _BUILT["nc"]


def kernel(x, w_qkv, b_qkv, w_proj, b_proj, _trace=None):
    from concourse import bass_utils

    in_maps = _host_inputs(x, w_qkv, b_qkv, w_proj, b_proj)
    nc = _get_nc()
    trace = bool(int(os.environ.get("TRACE_KERNEL", "0"))) if _trace is None else _trace
    res = bass_utils.run_bass_kernel_spmd(
        nc, in_maps, core_ids=list(range(NCORES)), trace=trace,
        trace_cores=list(range(NCORES)) if trace else None,
        stitch_traces=bool(trace),
    )
    _BUILT["last_results"] = res
    parts = [res.results[i]["out"] for i in range(NCORES)]
    out = np.empty((B, N, C), dtype=np.float32)
    for b in range(B):
        out[b] = parts[4 * b] + parts[4 * b + 1] + parts[4 * b + 2] + parts[4 * b + 3]
    return out



# revision 4
# speedup vs baseline: 1.1646x; 1.1646x over previous
"""Multi-head attention (B=2, N=4096, C=768, H=12, RoPE) on 8 trn2 NeuronCores.

Core c owns batch c//4, heads [(c%4)*3, +3). Software-pipelined: the ScalarE
exp stream and the TensorE matmul stream are kept busy concurrently; per-head
phases interleave via a paced background-unit queue.

Phase-1 restructure over the previous checkpoint (653908 ns -> target ~420us):
  - q and k projections merged into ONE 6-matmul chain per (head, block):
    the weight tile holds [q(64 cols, xscale*log2e) | k(64)], and the RoPE
    rotate-half duplicate rows of QT/KTt are produced by SBUF->SBUF DMA
    instead of duplicated weight columns (halves qk-projection PE time).
  - RoPE rotate-half reads the permuted head dims directly from the psum
    tile as two 32-partition STT ops (tab rows 64:96 = -sin[0:32],
    96:128 = +sin[32:64]).
  - exp chunks are [128, 1024] (2 k-tiles); PSUM = 2 chunks (4 banks)
    + double-buffered PV accumulator (2) + double-buffered work bank (2),
    removing the WAR stalls that inflated matmul durations (PE p-states:
    any PE gap drops the clock 2.4->1.2 GHz for ~3us).
  - V projection unpadded (192 free, not 256).
  - attention scale AND log2(e) are folded into the q weights/bias; exp is
    computed as exp(ln2 * y) via the activation's free scale multiplier
    (prep for splitting the exp stream onto the DVE).

HW gotcha (previous session): custom DVE ops silently return garbage on
hardware when an operand's base partition is nonzero - copy rows to
partition 0 first.
"""

import os
import sys

sys.path.insert(0, "/opt/trn_rl_repo")

import numpy as np
import ml_dtypes

B, N, C = 2, 4096, 768
H = 12
HD = 64
HH = HD // 2  # 32
THETA = 10000.0
NCORES = 8
HPC = 3  # heads per core
NT = N // 128  # 32 n-tiles
NBLK = N // 512  # 8 q blocks
KT = N // 128  # 32 k-tiles
LOG2E = float(np.log2(np.e))
LN2 = float(np.log(2.0))

BF16 = ml_dtypes.bfloat16

_BUILT = {}


def _rope_tables():
    inv = 1.0 / (THETA ** (np.arange(0, HD, 2, dtype=np.float64) / HD))  # [32]
    freqs = np.arange(N, dtype=np.float64)[:, None] * inv[None, :]  # [N, 32]
    cos = np.concatenate([np.cos(freqs), np.cos(freqs)], axis=-1)  # [N, 64]
    sin = np.concatenate([np.sin(freqs), np.sin(freqs)], axis=-1)
    cosT = cos.T.astype(np.float32)  # [64, N]
    sinT = sin.T.astype(np.float32)
    # sinx rows 0:32 = -sin rows 0:32 ; rows 32:64 = +sin rows 32:64
    sinxT = np.concatenate([-sinT[:HH], sinT[HH:]], axis=0)
    return cosT, sinxT


def _host_inputs(x, w_qkv, b_qkv, w_proj, b_proj):
    """Build the per-core input maps (all numpy, fp32/bf16)."""
    x = np.asarray(x, dtype=np.float32)
    w_qkv = np.asarray(w_qkv, dtype=np.float32)
    b_qkv = np.asarray(b_qkv, dtype=np.float32)
    w_proj = np.asarray(w_proj, dtype=np.float32)
    b_proj = np.asarray(b_proj, dtype=np.float32)

    cosT, sinxT = _rope_tables()
    qscale = (HD ** -0.5) * LOG2E  # attention scale + base-2 exp fold
    # STT operands must share their start partition, so the tables carry the
    # cos/sin rows pre-placed for both the q (rows 0:64) and k (64:128)
    # slices of the merged projection psum tile.
    F16 = np.float16
    tab_cos = np.concatenate([cosT, cosT], axis=0).astype(F16)  # [128, N]
    # per 32-row slot: [+sin[32:64]; -sin[0:32]] twice
    tab_sin = np.concatenate(
        [sinxT[HH:], sinxT[:HH], sinxT[HH:], sinxT[:HH]], axis=0
    ).astype(F16)  # [128, N]

    wT = w_qkv.T  # [C, 3C]  (c, e)
    wpT = w_proj.T  # [C, C]  (c, dd)

    in_maps = []
    for core in range(NCORES):
        b = core // 4
        h0 = (core % 4) * HPC
        xT = np.ascontiguousarray(x[b].T).astype(BF16)  # [C, N]

        # wqkT: per head ONE e-tile of 128 cols: [q(64, scaled); k(64)]
        etiles = []
        for h in range(h0, h0 + HPC):
            wq = wT[:, h * HD:(h + 1) * HD] * qscale  # [C, 64]
            wk = wT[:, C + h * HD: C + (h + 1) * HD]
            etiles.append(np.concatenate([wq, wk], axis=1))
        wqkT = np.ascontiguousarray(np.concatenate(etiles, axis=1)).astype(BF16)

        # v weights, unpadded 192 free
        wv = np.concatenate(
            [wT[:, 2 * C + h * HD: 2 * C + (h + 1) * HD] for h in range(h0, h0 + HPC)],
            axis=1,
        )  # [768, 192]
        wvT = np.ascontiguousarray(wv).astype(BF16)

        # merged qk bias: rows 0:64 = q bias (scaled), 64:128 = k bias
        bqk = np.zeros((128, HPC), dtype=np.float32)
        for j, h in enumerate(range(h0, h0 + HPC)):
            bqk[:HD, j] = b_qkv[h * HD:(h + 1) * HD] * qscale
            bqk[HD:, j] = b_qkv[C + h * HD: C + (h + 1) * HD]
        bv = np.zeros((128, 192), dtype=np.float32)
        for j, h in enumerate(range(h0, h0 + HPC)):
            bv[:, j * HD:(j + 1) * HD] = b_qkv[2 * C + h * HD: 2 * C + (h + 1) * HD]

        # proj weights: heads A,B stacked; head C + bias ones-row
        hA, hB, hC = h0, h0 + 1, h0 + 2
        wp_ab = np.concatenate(
            [wpT[hA * HD:(hA + 1) * HD], wpT[hB * HD:(hB + 1) * HD]], axis=0
        ).astype(BF16)  # [128, 768]
        wp_c = np.zeros((128, C), dtype=np.float32)
        wp_c[:HD] = wpT[hC * HD:(hC + 1) * HD]
        if core % 4 == 0:
            wp_c[HD] = b_proj  # bias once per batch (summed over 4 cores)
        wp_c = wp_c.astype(BF16)

        in_maps.append(
            {
                "xT": xT,
                "wqkT": wqkT,
                "wvT": wvT,
                "bqk": bqk,
                "bv": bv,
                "wp_ab": np.ascontiguousarray(wp_ab),
                "wp_c": np.ascontiguousarray(wp_c),
                "tab_cos": np.ascontiguousarray(tab_cos),
                "tab_sin": np.ascontiguousarray(tab_sin),
                "ones_f": np.ones((1, 64), dtype=np.float32),
            }
        )
    return in_maps


def _build_nc():
    import concourse.bass as bass  # noqa: F401
    import concourse.bacc as bacc
    import concourse.tile as tile
    import concourse.mybir as mybir

    f32 = mybir.dt.float32
    f16 = mybir.dt.float16
    bf16 = mybir.dt.bfloat16

    nc = bacc.Bacc("TRN2", num_devices=NCORES, debug=False)

    xT_d = nc.dram_tensor("xT", [C, N], bf16, kind="ExternalInput").ap()
    wqkT_d = nc.dram_tensor("wqkT", [C, HPC * 128], bf16, kind="ExternalInput").ap()
    wvT_d = nc.dram_tensor("wvT", [C, 192], bf16, kind="ExternalInput").ap()
    bqk_d = nc.dram_tensor("bqk", [128, HPC], f32, kind="ExternalInput").ap()
    bv_d = nc.dram_tensor("bv", [128, 192], f32, kind="ExternalInput").ap()
    wpab_d = nc.dram_tensor("wp_ab", [128, C], bf16, kind="ExternalInput").ap()
    wpc_d = nc.dram_tensor("wp_c", [128, C], bf16, kind="ExternalInput").ap()
    tabc_d = nc.dram_tensor("tab_cos", [128, N], f16, kind="ExternalInput").ap()
    tabs_d = nc.dram_tensor("tab_sin", [128, N], f16, kind="ExternalInput").ap()
    onesf_d = nc.dram_tensor("ones_f", [1, 64], f32, kind="ExternalInput").ap()
    out_d = nc.dram_tensor("out", [N, C], f32, kind="ExternalOutput").ap()
    taps = {}
    if os.environ.get("KTAPS") == "1":
        taps["qt"] = nc.dram_tensor("tap_qt", [128, N], bf16, kind="ExternalOutput").ap()
        taps["kt"] = nc.dram_tensor("tap_kt", [128, N], bf16, kind="ExternalOutput").ap()
        taps["v"] = nc.dram_tensor("tap_v", [128, NT * HPC * (HD + 1)], bf16, kind="ExternalOutput").ap()
        taps["sp"] = nc.dram_tensor("tap_sp", [128, N], bf16, kind="ExternalOutput").ap()
        taps["sc"] = nc.dram_tensor("tap_sc", [128, N], bf16, kind="ExternalOutput").ap()
        taps["e"] = nc.dram_tensor("tap_e", [128, 1024], bf16, kind="ExternalOutput").ap()
        taps["z"] = nc.dram_tensor("tap_z", [128, 512], f32, kind="ExternalOutput").ap()

    with tile.TileContext(nc) as tc:
        _emit(tc, nc, mybir, locals())
    nc.compile()
    return nc


def _emit(tc, nc, mybir, T):
    taps = T["taps"]
    f32 = mybir.dt.float32
    f32r = mybir.dt.float32r
    f16 = mybir.dt.float16
    bf16 = mybir.dt.bfloat16
    ALU = mybir.AluOpType
    EXP = mybir.ActivationFunctionType.Exp

    xT_d = T["xT_d"]; wqkT_d = T["wqkT_d"]; wvT_d = T["wvT_d"]
    bqk_d = T["bqk_d"]; bv_d = T["bv_d"]
    wpab_d = T["wpab_d"]; wpc_d = T["wpc_d"]
    tabc_d = T["tabc_d"]; tabs_d = T["tabs_d"]
    onesf_d = T["onesf_d"]; out_d = T["out_d"]

    from contextlib import ExitStack

    ctx = ExitStack()
    with ctx:
        const = ctx.enter_context(tc.tile_pool(name="const", bufs=1))
        ropes = ctx.enter_context(tc.tile_pool(name="ropes", bufs=2))
        norms = ctx.enter_context(tc.tile_pool(name="norms", bufs=2))
        epool = ctx.enter_context(tc.tile_pool(name="epool", bufs=4))
        fout = ctx.enter_context(tc.tile_pool(name="fout", bufs=2))
        hw = ctx.enter_context(tc.tile_pool(name="hw", bufs=1))
        # PSUM: 2 chunks (4 banks) + 2 pv accumulators + 2 work banks
        scps = ctx.enter_context(tc.tile_pool(name="scps", bufs=2, space="PSUM"))
        pvps = ctx.enter_context(tc.tile_pool(name="pvps", bufs=2, space="PSUM"))
        wkps = ctx.enter_context(tc.tile_pool(name="wkps", bufs=2, space="PSUM"))

        # ---- constants / resident tensors ----
        xT = const.tile([128, 6, N], bf16, tag="xT")
        for ct in range(6):
            nc.sync.dma_start(out=xT[:, ct, :], in_=xT_d[ct * 128:(ct + 1) * 128, :])
        tab_cos = const.tile([128, N], f16, tag="tab_cos")
        nc.sync.dma_start(out=tab_cos, in_=tabc_d)
        tab_sin = const.tile([128, N], f16, tag="tab_sin")
        nc.sync.dma_start(out=tab_sin, in_=tabs_d)
        wvT = const.tile([128, 6, 192], bf16, tag="wvT")
        nc.sync.dma_start(out=wvT, in_=wvT_d.rearrange("(t p) e -> p t e", p=128))
        bv = const.tile([128, 192], f32, tag="bv")
        nc.sync.dma_start(out=bv, in_=bv_d)
        bqk = const.tile([128, HPC], f32, tag="bqk")
        nc.sync.dma_start(out=bqk, in_=bqk_d)
        wp_ab = const.tile([128, C], bf16, tag="wp_ab")
        nc.sync.dma_start(out=wp_ab, in_=wpab_d)
        wp_c = const.tile([128, C], bf16, tag="wp_c")
        nc.sync.dma_start(out=wp_c, in_=wpc_d)
        ones_f = const.tile([1, 64], f32, tag="ones_f")
        nc.sync.dma_start(out=ones_f, in_=onesf_d)
        ones_r = const.tile([1, 64], f32r, tag="ones_r")
        nc.vector.tensor_copy(ones_r, ones_f)

        # double-buffered per-head V (ones column at [..., 64] -> softmax
        # denominators free in PSUM row 64 of the PV accumulation)
        V_all = const.tile([128, 2, NT, HPC, HD + 1], bf16, tag="V_all")
        for vb in range(2):
            for j in range(HPC):
                nc.vector.memset(V_all[:, vb, :, j, HD:HD + 1], 1.0)
        stagingP = const.tile([128, N], bf16, tag="stagingP")
        stagingC = const.tile([128, N], bf16, tag="stagingC")
        nc.vector.memset(stagingC[HD:HD + 1, :], 1.0)  # proj-bias ones row

        # double-buffered Q^T/K^T (rows 64:128 duplicate rows 0:64 via DMA)
        QT = const.tile([128, 2, N], bf16, tag="QT")
        KTt = const.tile([128, 2, N], bf16, tag="KTt")

        wqk_bufs = [
            hw.tile([128, 6, 128], bf16, tag="wqk", name="wqk0"),
            hw.tile([128, 6, 128], bf16, tag="wqk2", name="wqk1"),
        ]

        # ---------------- unit emitters ----------------
        def unit_wqk_load(gh):
            """DMA the merged q|k weights for global-head gh into its buffer."""
            h = gh % HPC
            nc.sync.dma_start(
                out=wqk_bufs[gh % 2],
                in_=wqkT_d.rearrange("(t p) e -> p t e", p=128)[
                    :, :, h * 128:(h + 1) * 128
                ],
            )

        def unit_v(vb, nt):
            """V projection for n-tile nt into V buffer vb."""
            vp = wkps.tile([128, 512], f32, tag="wk", name="vp")
            for ct in range(6):
                nc.tensor.matmul(
                    vp[:, 0:192],
                    lhsT=xT[:, ct, nt * 128:(nt + 1) * 128],
                    rhs=wvT[:, ct, :],
                    start=(ct == 0),
                    stop=(ct == 5),
                )
            nc.vector.scalar_tensor_tensor(
                out=V_all[:, vb, nt, :, 0:HD],
                in0=vp[:, 0:192].rearrange("p (a b) -> p a b", a=HPC),
                scalar=1.0,
                in1=bv.rearrange("p (a b) -> p a b", a=HPC),
                op0=ALU.mult,
                op1=ALU.add,
            )

        def unit_qk(gh, blk):
            """Merged q+k projection + RoPE for (head gh, block blk)."""
            h = gh % HPC
            hb = gh % 2
            wqk = wqk_bufs[gh % 2]
            ns = slice(blk * 512, (blk + 1) * 512)
            qp = wkps.tile([128, 512], f32, tag="wk", name="qp")
            for ct in range(6):
                nc.tensor.matmul(
                    qp,
                    lhsT=wqk[:, ct, :],
                    rhs=xT[:, ct, ns],
                    start=(ct == 0),
                    stop=(ct == 5),
                )
            for et, dest in ((0, QT), (1, KTt)):
                b0 = et * HD
                # rotate-half part: rows 0:32 <- -(q[32:64]+b)sin[0:32],
                # rows 32:64 <- (q[0:32]+b)sin[32:64]
                t_s = ropes.tile([HD, 512], f32, tag="t_s")
                nc.vector.scalar_tensor_tensor(
                    out=t_s[0:HH, :],
                    in0=qp[b0 + HH:b0 + HD, :],
                    scalar=bqk[b0 + HH:b0 + HD, h:h + 1],
                    in1=tab_sin[b0 + HH:b0 + HD, ns],
                    op0=ALU.add,
                    op1=ALU.mult,
                )
                nc.vector.scalar_tensor_tensor(
                    out=t_s[HH:HD, :],
                    in0=qp[b0:b0 + HH, :],
                    scalar=bqk[b0:b0 + HH, h:h + 1],
                    in1=tab_sin[b0:b0 + HH, ns],
                    op0=ALU.add,
                    op1=ALU.mult,
                )
                u_s = ropes.tile([HD, 512], f32, tag="u_s")
                nc.vector.scalar_tensor_tensor(
                    out=u_s,
                    in0=qp[b0:b0 + HD, :],
                    scalar=bqk[b0:b0 + HD, h:h + 1],
                    in1=tab_cos[b0:b0 + HD, ns],
                    op0=ALU.add,
                    op1=ALU.mult,
                )
                nc.vector.tensor_add(dest[0:HD, hb, ns], t_s, u_s)
                # duplicate rows for the PE row-group pairing, off the DVE
                nc.sync.dma_start(
                    out=dest[HD:128, hb, ns], in_=dest[0:HD, hb, ns]
                )

        def unit_out(nt):
            """Output projection for n-tile nt (plus DMA)."""
            nsl = slice(nt * 128, (nt + 1) * 128)
            fs = fout.tile([128, C], f32, tag="fs")
            for o, w in ((0, 512), (512, 256)):
                fp = wkps.tile([128, 512], f32, tag="wk", name="fp")
                nc.tensor.matmul(
                    fp[:, 0:w], lhsT=stagingP[:, nsl], rhs=wp_ab[:, o:o + w],
                    start=True, stop=False,
                )
                nc.tensor.matmul(
                    fp[:, 0:w], lhsT=stagingC[0:HD + 1, nsl],
                    rhs=wp_c[0:HD + 1, o:o + w],
                    start=False, stop=True,
                )
                nc.vector.tensor_copy(fs[:, o:o + w], fp[:, 0:w])
            nc.sync.dma_start(out=out_d[nsl, :], in_=fs)

        # paced background-unit emission
        pending = []

        def pump(frac_done):
            """Emit pending units up to frac_done of the current window."""
            target = int(np.ceil(frac_done * pump.total))
            while pump.emitted < target and pending:
                pending.pop(0)()
                pump.emitted += 1

        def set_window():
            pump.total = max(len(pending), 1)
            pump.emitted = 0

        # ---------------- attention pass ----------------
        deferred = []

        def flush_deferred():
            while deferred:
                deferred.pop(0)()

        def norm_tail(gh, qb, pvraw, tap_ok):
            h = gh % HPC
            q0 = slice(qb * 512, (qb + 1) * 512)
            # align the denominator row to partition 0 first: custom DVE ops
            # do not handle partition-base-shifted operands on HW
            zc = norms.tile([1, 512], f32, tag="zc")
            nc.vector.tensor_copy(zc, pvraw[HD:HD + 1, :])
            r0 = norms.tile([1, 512], f32, tag="r0")
            nc.vector.reciprocal_approx_fast(out=r0, in_=zc)
            r0r = norms.tile([1, 512], f32r, tag="r0r")
            nc.vector.tensor_copy(r0r, r0)
            rb = wkps.tile([128, 512], f32, tag="wk", name="rb")
            # f32r operands -> full-rate broadcast matmul (512 free >= 256)
            nc.tensor.matmul(
                rb[0:HD, :], lhsT=ones_r, rhs=r0r, start=True, stop=True,
            )
            rbs = norms.tile([HD, 512], f32, tag="rbs")
            nc.vector.tensor_copy(rbs, rb[0:HD, :])
            if tap_ok and qb == 0 and "z" in taps:
                zt = norms.tile([128, 512], f32, tag="zt")
                nc.vector.memset(zt, 0.0)
                nc.vector.tensor_copy(zt[0:1, :], pvraw[HD:HD + 1, :])
                nc.sync.dma_start(out=taps["z"], in_=zt)
            if h == 0:
                d0 = stagingP[0:HD, q0]
            elif h == 1:
                d0 = stagingP[HD:128, q0]
            else:
                d0 = stagingC[0:HD, q0]
            nc.vector.tensor_mul(d0, pvraw[0:HD, :], rbs)
            if h == HPC - 1:
                for nt in range(4 * qb, 4 * qb + 4):
                    unit_out(nt)

        def attn_pass(gh, qb, tap_ok):
            """One 512-wide q-block of softmax attention for head gh."""
            h = gh % HPC
            hb = gh % 2
            vb = (gh // HPC) % 2
            q0 = slice(qb * 512, (qb + 1) * 512)
            pvb = pvps.tile([128, 512], f32, tag="pv", name="pvb")
            nchunks = KT // 2  # 16 chunks of 2 k-tiles
            for ci in range(nchunks):
                kts = (2 * ci, 2 * ci + 1)
                sc = scps.tile([128, 1024], f32, tag="sc", name="sc")
                nc.tensor.matmul(
                    sc[:, 0:512],
                    lhsT=KTt[0:HD, hb, kts[0] * 128:(kts[0] + 1) * 128],
                    rhs=QT[0:HD, hb, q0],
                    start=True, stop=True,
                )
                nc.tensor.matmul(
                    sc[:, 512:1024],
                    lhsT=KTt[HD:128, hb, kts[1] * 128:(kts[1] + 1) * 128],
                    rhs=QT[HD:128, hb, q0],
                    start=True, stop=True, tile_position=(64, 0),
                )
                ec = epool.tile([128, 1024], bf16, tag="ec")
                nc.scalar.activation(ec, sc, EXP, scale=LN2)
                if tap_ok and ci == 0 and "e" in taps:
                    nc.sync.dma_start(out=taps["e"], in_=ec)
                for j, kt in enumerate(kts):
                    nc.tensor.matmul(
                        pvb[0:HD + 1, :],
                        lhsT=V_all[:, vb, kt, h, :],
                        rhs=ec[:, j * 512:(j + 1) * 512],
                        start=(kt == 0), stop=(kt == KT - 1),
                    )
                if ci == 2:
                    flush_deferred()
                pump((qb * nchunks + ci + 1) / (NBLK * nchunks))
            # drain pvb with a raw copy; norm chain runs early next pass
            pvraw = norms.tile([HD + 1, 512], f32, tag="pvraw")
            nc.vector.tensor_copy(pvraw, pvb[0:HD + 1, :])
            deferred.append(lambda: norm_tail(gh, qb, pvraw, tap_ok))

        # ---------------- program ----------------
        NREP = int(os.environ.get("KREPEAT", "1"))

        # cold start: weights for head 0, all qk blocks, V buffer 0
        unit_wqk_load(0)
        for blk in range(NBLK):
            unit_qk(0, blk)
        for nt in range(NT):
            unit_v(0, nt)
        unit_wqk_load(1)

        for rep in range(NREP):
            for h in range(HPC):
                gh = rep * HPC + h
                # fill the background queue for this head's window
                pending.clear()
                if h == 0:
                    for blk in range(NBLK):
                        pending.append(lambda g=gh + 1, b=blk: unit_qk(g, b))
                    pending.append(lambda g=gh + 2: unit_wqk_load(g))
                elif h == 1:
                    for blk in range(NBLK):
                        pending.append(lambda g=gh + 1, b=blk: unit_qk(g, b))
                    pending.append(lambda g=gh + 2: unit_wqk_load(g))
                    if rep + 1 < NREP:
                        # next rep's V projection (double-buffered)
                        vb1 = (rep + 1) % 2
                        for nt in range(NT):
                            pending.append(lambda v=vb1, n=nt: unit_v(v, n))
                else:
                    # last head: next rep's head-0 projections; the output
                    # projection interleaves per-q-block below.
                    if rep + 1 < NREP:
                        g1 = (rep + 1) * HPC
                        for blk in range(NBLK):
                            pending.append(lambda g=g1, b=blk: unit_qk(g, b))
                        pending.append(lambda g=g1 + 1: unit_wqk_load(g))
                set_window()

                for qb in range(NBLK):
                    attn_pass(gh, qb, tap_ok=(rep == 0 and h == 0))

            if rep == NREP - 1 and h == HPC - 1:
                flush_deferred()
            if rep == 0 and "qt" in taps:
                nc.sync.dma_start(out=taps["qt"], in_=QT[:, 0, :])
                nc.sync.dma_start(out=taps["kt"], in_=KTt[:, 0, :])
            if rep == 0 and "v" in taps:
                nc.sync.dma_start(
                    out=taps["v"], in_=V_all[:, 0].rearrange("p a b c -> p (a b c)")
                )
            if rep == 0 and "sp" in taps:
                nc.sync.dma_start(out=taps["sp"], in_=stagingP)
                nc.sync.dma_start(out=taps["sc"], in_=stagingC)


def _get_nc():
    if "nc" not in _BUILT:
        _BUILT["nc"] = _build_nc()
    return _BUILT["nc"]


def kernel(x, w_qkv, b_qkv, w_proj, b_proj, _trace=None):
    from concourse import bass_utils

    in_maps = _host_inputs(x, w_qkv, b_qkv, w_proj, b_proj)
    nc = _get_nc()
    trace = bool(int(os.environ.get("TRACE_KERNEL", "0"))) if _trace is None else _trace
    res = bass_utils.run_bass_kernel_spmd(
        nc, in_maps, core_ids=list(range(NCORES)), trace=trace,
        trace_cores=list(range(NCORES)) if trace else None,
        stitch_traces=bool(trace),
    )
    _BUILT["last_results"] = res
    parts = [res.results[i]["out"] for i in range(NCORES)]
    out = np.empty((B, N, C), dtype=np.float32)
    for b in range(B):
        out[b] = parts[4 * b] + parts[4 * b + 1] + parts[4 * b + 2] + parts[4 * b + 3]
    return out
